# revision 38
# baseline (speedup 1.0000x reference)
"""ContentAttention kernel for 8 Trainium2 NeuronCores (v3).

Computation (per batch b):
    h_att  = h[b] @ W_h2att + b_h2att                  # [512]
    e      = tanh(p_att_feats[b] + h_att)              # [1024, 512]
    scores = e @ w_alpha (+ b_alpha, dropped: softmax shift-invariant)
    w      = softmax(scores)                           # [1024]
    out[b] = w @ att_feats[b]                          # [1024]

Sharding: data-parallel over batch B=128 -> 16 batches/core x 8 cores.
Params are tiny and replicated.

Design notes (evidence-driven; v1 was DVE/ACT/DMA 3-way bound at ~180us):
  - Aggregate HBM->SBUF DMA tops out at ~320-340 GB/s SBUF-side no
    matter how many queues are used (measured), so wall time is floored
    by SBUF-side DMA bytes. Everything below minimizes those bytes and
    keeps the single big stream in consumption order.
  - p_att_feats is host-transposed to [D, R] per batch (d on partitions)
    and quantized int8 (4-sigma clip): ACT reads int8 directly and fuses
    the h_att add via the per-partition bias AP: e = tanh(s*p + h_att).
    This deletes v1's DVE add/mul/reduce (DVE 131us -> 26us).
  - scores = e @ w_alpha moves to PE: contraction over d lies along
    partitions; 32 FWL matmuls with contiguous e-chunks stationary
    produce scores directly as [128, 8] (r = 8*rp + j), matching att's
    layout. No transposes, no partition reduction.
  - softmax normalization deferred to the output: out = (sum exp(s)*att)
    * s_att / Z; Z via exp's accum_out + a tiny ones-matmul. Weights are
    unnormalized exp in bf16 -> no gpsimd all-reduce, no extra DVE pass.
  - att_feats: int8 in HBM, SWDGE cast to bf16 during the DMA (cast is
    free: same SBUF-side rate as plain bf16), packed 2 batches per slab
    so each partition is one 16KB contiguous run; the last two batches
    are single-batch slabs so the tail compute overlaps final bytes.
  - ALL big streams (W, p, att) ride ONE gpsimd/SWDGE queue in exact
    consumption order: a single always-fed queue sustains ~395 GB/s
    while two queues round-robining the same 16 SDMA engines only hit
    ~320-330 aggregate (measured both ways; the split cost 18us).
  - p packed 4 batches/slab (16KB int8 runs); 4KB runs ran at ~12GB/s.
  - small consts host-packed per-partition-contiguous: HWDGE descriptor
    generation is ~30ns/desc; a [128,8,16] gather AP cost 32us of issue.
  - total rel err ~1.1e-2 (numpy-simulated exactly): int8 p ~0.6%,
    int8 att ~0.96%, bf16 weights/e ~0.3%; gate is 2e-2.

Per-iteration engine streams (6-stage skewed software pipeline; exp sits
two iterations behind scores so ACT, which runs ~2us/iter faster than
PE, never blocks the tanh queued behind it):
    ACT:  4x tanh(b), exp(b-3)
    PE:   Z(b-4), 16x wsum(b-4), 32x scores(b-1)
    DVE:  recip(b-4), 2x out-scale(b-5)
    gpsimd: p-group + att-slab cast-DMA issues (consumption order)
    sync: consts, out store (b-5)
"""

import numpy as np

B, R, K_H, D, F = 128, 1024, 1024, 512, 1024
N_CORES = 8
BPC = B // N_CORES  # batches per core
RC = R // 128  # region chunks (r = 8*rp + j)
DC = D // 128  # d chunks (d = 4*pp + dc)
KC = K_H // 128
S_P = 4.0 / 127.0  # p_att_feats int8 scale
S_A = 4.0 / 127.0  # att_feats int8 scale
AB = 2  # batches per att slab
PB = 4  # batches per p slab

_cached = {}


def _build_program():
    from contextlib import ExitStack

    import concourse.bass as bass
    import concourse.tile as tile
    from concourse import bacc, mybir

    f32 = mybir.dt.float32
    bf16 = mybir.dt.bfloat16
    i8 = mybir.dt.int8
    AF = mybir.ActivationFunctionType
    ALU = mybir.AluOpType

    nc = bacc.Bacc("TRN2", target_bir_lowering=False, debug=False)
    p_ap = nc.dram_tensor(
        "p", [BPC // PB, 128, DC * PB * R], i8, kind="ExternalInput"
    ).ap()
    att_ap = nc.dram_tensor(
        "att", [128, BPC * RC * F], i8, kind="ExternalInput"
    ).ap()
    w_ap = nc.dram_tensor("w_h2att", [128, KC * D], bf16, kind="ExternalInput").ap()
    ht_ap = nc.dram_tensor("ht", [128, KC * BPC], bf16, kind="ExternalInput").ap()
    cst_ap = nc.dram_tensor("csts", [128, 24], mybir.dt.uint8, kind="ExternalInput").ap()
    out_ap = nc.dram_tensor("out", [BPC, F], bf16, kind="ExternalOutput").ap()

    with tile.TileContext(nc) as tc, ExitStack() as ctx:
        consts = ctx.enter_context(tc.tile_pool(name="consts", bufs=1))
        ppool = ctx.enter_context(tc.tile_pool(name="ppool", bufs=2))
        epool = ctx.enter_context(tc.tile_pool(name="epool", bufs=2))
        apool = ctx.enter_context(tc.tile_pool(name="apool", bufs=3))
        spool = ctx.enter_context(tc.tile_pool(name="spool", bufs=6))
        outp = ctx.enter_context(tc.tile_pool(name="outp", bufs=2))
        ps_mis = ctx.enter_context(tc.tile_pool(name="ps_mis", bufs=3, space="PSUM"))
        ps_out = ctx.enter_context(tc.tile_pool(name="ps_out", bufs=2, space="PSUM"))
        ps_z = ctx.enter_context(tc.tile_pool(name="ps_z", bufs=1, space="PSUM"))

        p_tiles, a_tiles = {}, {}

        def issue_p(g):
            # [128(pp), 4(dc), PB(k), 1024(col)]; d = 4*pp+dc, col = j*128+rp
            t = ppool.tile([128, DC, PB, R], i8, tag="p")
            nc.gpsimd.dma_start(out=t, in_=p_ap[g])
            p_tiles[g] = t

        # variable-width slabs: 2-batch mid-stream (16KB int8 runs, best
        # rate), single batches at the end so tail compute overlaps the
        # last arriving bytes
        SL = [(0, 3), (3, 3), (6, 3), (9, 3), (12, 2), (14, 1), (15, 1)]
        b2slab = {}
        for si, (s0, nb) in enumerate(SL):
            for k in range(nb):
                b2slab[s0 + k] = (si, k)

        def issue_a(si):
            s0, nb = SL[si]
            # int8 HBM -> bf16 SBUF cast during the DMA (SWDGE only)
            t = apool.tile([128, nb, RC, F], bf16, tag="a", padded_shape=[128, 3, RC, F])
            nc.gpsimd.dma_start(
                out=t, in_=att_ap[:, s0 * RC * F : (s0 + nb) * RC * F].rearrange(
                    "p (k j f) -> p k j f", k=nb, j=RC
                )
            )
            a_tiles[si] = t

        # W first on the gpsimd queue: FIFO means it fully drains before
        # the att slabs instead of round-robining behind the prefetch burst.
        w_flat = consts.tile([128, KC * D], bf16, tag="w")
        nc.gpsimd.dma_start(out=w_flat, in_=w_ap)
        issue_p(0)
        cst_t = consts.tile([128, 24], mybir.dt.uint8, tag="csts")
        nc.sync.dma_start(out=cst_t, in_=cst_ap)
        wa_t = cst_t[:, 0:8].bitcast(bf16)  # [128, 4]
        b2_t = cst_t[:, 8:24].bitcast(f32)  # [128, 4]
        ht_flat = consts.tile([128, KC * BPC], bf16, tag="ht")
        nc.sync.dma_start(out=ht_flat, in_=ht_ap)
        issue_p(1)
        for g in range(3):
            issue_a(g)
        ones_t = consts.tile([128, 1], f32, tag="ones")
        nc.vector.memset(ones_t, 1.0)

        # ---- phase 0: h_attT[d, b] = (h @ W + b2).T, kept [128, 4, 16] f32
        hattT = consts.tile([128, DC, BPC], f32, tag="hatt")
        for mc in range(DC):
            ps_h = ps_mis.tile([128, BPC], f32, tag="mis")
            for kc in range(KC):
                nc.tensor.matmul(
                    ps_h,
                    lhsT=w_flat[:, kc * D + mc * 128 : kc * D + (mc + 1) * 128],
                    rhs=ht_flat[:, kc * BPC : (kc + 1) * BPC],
                    start=(kc == 0),
                    stop=(kc == KC - 1),
                )
            nc.scalar.activation(
                hattT[:, mc, :], ps_h, AF.Identity, bias=b2_t[:, mc : mc + 1]
            )

        # ---- main loop: 5-stage skewed software pipeline
        e_t, sc_t, ex_t, s1_t, z_t, rz_t, po_t = {}, {}, {}, {}, {}, {}, {}
        for i in range(BPC + 5):
            # T1 (ACT): tanh for b, first in the ACT stream (its inputs are
            # ready at iteration start; exp(b-2) depends on PE's tail)
            if i < BPC:
                b = i
                p_t = p_tiles[b // PB]
                e_tt = epool.tile([128, DC, R], bf16, tag="e")
                for dc in range(DC):
                    nc.scalar.activation(
                        e_tt[:, dc, :],
                        p_t[:, dc, b % PB, :],
                        AF.Tanh,
                        bias=hattT[:, dc, b : b + 1],
                        scale=S_P,
                    )
                if b % PB == PB - 1:
                    # next p group: its slot's last consumer just ran here
                    p_tiles.pop(b // PB)
                    if b // PB + 2 < BPC // PB:
                        issue_p(b // PB + 2)
                e_t[b] = e_tt

            # T3 (ACT): exp of scores(b-3) — two iterations of slack after
            # PE's scores so ACT never blocks even when it drifts ahead
            if 0 <= i - 3 < BPC:
                b = i - 3
                expb = spool.tile([128, RC], bf16, tag="expb")
                s1 = spool.tile([128, 1], f32, tag="s1")
                nc.scalar.activation(expb, sc_t.pop(b), AF.Exp, accum_out=s1)
                ex_t[b], s1_t[b] = expb, s1

            # T4 (PE): Z partition-sum + weighted sum for b-4
            if 0 <= i - 4 < BPC:
                b = i - 4
                ps_zz = ps_z.tile([1, 1], f32, tag="z")
                nc.tensor.matmul(
                    ps_zz, lhsT=s1_t.pop(b), rhs=ones_t, start=True, stop=True
                )
                z_t[b] = ps_zz
                si, kk = b2slab[b]
                a_tt = a_tiles[si]
                expb = ex_t.pop(b)
                ps0 = ps_out.tile([1, 512], f32, tag="ps0")
                ps1 = ps_out.tile([1, 512], f32, tag="ps1")
                for j in range(RC):
                    nc.tensor.matmul(
                        ps0,
                        lhsT=expb[:, j : j + 1],
                        rhs=a_tt[:, kk, j, 0:512],
                        start=(j == 0),
                        stop=(j == RC - 1),
                    )
                    nc.tensor.matmul(
                        ps1,
                        lhsT=expb[:, j : j + 1],
                        rhs=a_tt[:, kk, j, 512:1024],
                        start=(j == 0),
                        stop=(j == RC - 1),
                    )
                po_t[b] = (ps0, ps1)
                if kk == SL[si][1] - 1:
                    # slab finished: refill its slot in consumption order
                    a_tiles.pop(si)
                    if si + 3 < len(SL):
                        issue_a(si + 3)

            # T5 (DVE): 1/Z for b-4; out-scale + store for b-5
            if 0 <= i - 4 < BPC:
                b = i - 4
                rz = spool.tile([1, 1], f32, tag="rz")
                nc.vector.reciprocal(rz, z_t.pop(b))
                rz_t[b] = rz
            if 0 <= i - 5 < BPC:
                b = i - 5
                ps0, ps1 = po_t.pop(b)
                rz = rz_t.pop(b)
                ob = outp.tile([1, F], bf16)
                nc.vector.tensor_scalar(
                    ob[:, 0:512], ps0, rz, S_A, op0=ALU.mult, op1=ALU.mult
                )
                nc.vector.tensor_scalar(
                    ob[:, 512:1024], ps1, rz, S_A, op0=ALU.mult, op1=ALU.mult
                )
                nc.sync.dma_start(out=out_ap[b : b + 1, :], in_=ob)

            # T2 (PE): scores(b-1) as [128(rp), 8(j)] via stationary-e matmuls
            if 0 <= i - 1 < BPC:
                b = i - 1
                e_tt = e_t.pop(b)
                ps_s = ps_mis.tile([128, RC], f32, tag="mis")
                for j in range(RC):
                    for dc in range(DC):
                        nc.tensor.matmul(
                            ps_s[:, j : j + 1],
                            lhsT=e_tt[:, dc, j * 128 : (j + 1) * 128],
                            rhs=wa_t[:, dc : dc + 1],
                            start=(dc == 0),
                            stop=(dc == DC - 1),
                        )
                sc_t[b] = ps_s

    nc.compile()
    return nc


def _get_program():
    if "nc" not in _cached:
        _cached["nc"] = _build_program()
    return _cached["nc"]


def _make_in_maps(inputs):
    import ml_dtypes

    bf = ml_dtypes.bfloat16
    h = np.asarray(inputs["h"], dtype=np.float32)
    att = np.asarray(inputs["att_feats"], dtype=np.float32)
    p = np.asarray(inputs["p_att_feats"], dtype=np.float32)
    W = np.asarray(inputs["W_h2att"], dtype=np.float32)
    b2 = np.asarray(inputs["b_h2att"], dtype=np.float32)
    wa = np.asarray(inputs["w_alpha"], dtype=np.float32)
    # b_alpha is a scalar added to every score; softmax is shift-invariant.

    # p: [B, R, D] -> int8 [B, D, R'] with r' = j*128 + rp for r = 8*rp + j
    pq = np.clip(np.rint(p * (1.0 / S_P)), -127, 127).astype(np.int8)
    pT = np.ascontiguousarray(
        pq.reshape(B, 128, RC, D).transpose(0, 3, 2, 1)
    ).reshape(B, D, R)
    # att: int8, 2-batch slabs, per-partition contiguous [g, 128, 2*8*1024]
    aq = np.clip(np.rint(att * (1.0 / S_A)), -127, 127).astype(np.int8)
    # W columns permuted so matmul chunk mc / partition pp -> d = 4*pp + mc,
    # then packed per-partition: W_pack[pk, kc*D + m] = W_perm[kc*128+pk, m]
    Wp = np.ascontiguousarray(
        W.reshape(K_H, 128, DC).transpose(0, 2, 1)
    ).reshape(K_H, D).astype(bf)
    W_pack = np.ascontiguousarray(
        Wp.reshape(KC, 128, D).transpose(1, 0, 2)
    ).reshape(128, KC * D)
    hT = np.ascontiguousarray(h.T).astype(bf)  # [K_H, B]
    wa_t = np.ascontiguousarray(wa.reshape(128, DC)).astype(bf)
    b2_t = np.ascontiguousarray(b2.reshape(128, DC)).astype(np.float32)
    csts = np.ascontiguousarray(
        np.concatenate([wa_t.view(np.uint8), b2_t.view(np.uint8)], axis=1)
    )  # [128, 24]

    in_maps = []
    for c in range(N_CORES):
        lo, hi = c * BPC, (c + 1) * BPC
        # p groups of PB batches: [g, 128, DC*PB*R] per-partition contiguous
        p4 = np.ascontiguousarray(
            pT[lo:hi]
            .reshape(BPC // PB, PB, 128, DC, R)  # d = pp*4 + dc
            .transpose(0, 2, 3, 1, 4)  # [g, pp, dc, k, R]
        ).reshape(BPC // PB, 128, DC * PB * R)
        # att flat: [128, BPC*RC*F]; per partition batch-major, r = 8*rp+j
        a2 = np.ascontiguousarray(
            aq[lo:hi].reshape(BPC, 128, RC * F).transpose(1, 0, 2)
        ).reshape(128, BPC * RC * F)
        # ht_pack[pk, kc*BPC + b] = h[lo+b, kc*128+pk]
        ht_pack = np.ascontiguousarray(
            hT[:, lo:hi].reshape(KC, 128, BPC).transpose(1, 0, 2)
        ).reshape(128, KC * BPC)
        in_maps.append(
            {
                "p": p4,
                "att": a2,
                "w_h2att": W_pack,
                "ht": ht_pack,
                "csts": csts,
            }
        )
    return in_maps


def kernel(**inputs) -> np.ndarray:
    from concourse.bass_utils import run_bass_kernel_spmd

    nc = _get_program()
    in_maps = _make_in_maps(inputs)
    res = run_bass_kernel_spmd(nc, in_maps, list(range(N_CORES)))
    out = np.concatenate([res.results[c]["out"] for c in range(N_CORES)], axis=0)
    return out.astype(np.float32)


# revision 39
# speedup vs baseline: 1.0791x; 1.0791x over previous
"""ContentAttention kernel for 8 Trainium2 NeuronCores (v3).

Computation (per batch b):
    h_att  = h[b] @ W_h2att + b_h2att                  # [512]
    e      = tanh(p_att_feats[b] + h_att)              # [1024, 512]
    scores = e @ w_alpha (+ b_alpha, dropped: softmax shift-invariant)
    w      = softmax(scores)                           # [1024]
    out[b] = w @ att_feats[b]                          # [1024]

Sharding: data-parallel over batch B=128 -> 16 batches/core x 8 cores.
Params are tiny and replicated.

Design notes (evidence-driven; v1 was DVE/ACT/DMA 3-way bound at ~180us):
  - Aggregate HBM->SBUF DMA tops out at ~320-340 GB/s SBUF-side no
    matter how many queues are used (measured), so wall time is floored
    by SBUF-side DMA bytes. Everything below minimizes those bytes and
    keeps the single big stream in consumption order.
  - p_att_feats is host-transposed to [D, R] per batch (d on partitions)
    and quantized int8 (4-sigma clip): ACT reads int8 directly and fuses
    the h_att add via the per-partition bias AP: e = tanh(s*p + h_att).
    This deletes v1's DVE add/mul/reduce (DVE 131us -> 26us).
  - scores = e @ w_alpha moves to PE: contraction over d lies along
    partitions; 32 FWL matmuls with contiguous e-chunks stationary
    produce scores directly as [128, 8] (r = 8*rp + j), matching att's
    layout. No transposes, no partition reduction.
  - softmax normalization deferred to the output: out = (sum exp(s)*att)
    * s_att / Z; Z via exp's accum_out + a tiny ones-matmul. Weights are
    unnormalized exp in bf16 -> no gpsimd all-reduce, no extra DVE pass.
  - att_feats: int8 in HBM, SWDGE cast to bf16 during the DMA (cast is
    free: same SBUF-side rate as plain bf16), packed 2 batches per slab
    so each partition is one 16KB contiguous run; the last two batches
    are single-batch slabs so the tail compute overlaps final bytes.
  - ALL big streams (W, p, att) ride ONE gpsimd/SWDGE queue in exact
    consumption order: a single always-fed queue sustains ~395 GB/s
    while two queues round-robining the same 16 SDMA engines only hit
    ~320-330 aggregate (measured both ways; the split cost 18us).
  - p packed 4 batches/slab (16KB int8 runs); 4KB runs ran at ~12GB/s.
  - small consts host-packed per-partition-contiguous: HWDGE descriptor
    generation is ~30ns/desc; a [128,8,16] gather AP cost 32us of issue.
  - total rel err ~1.1e-2 (numpy-simulated exactly): int8 p ~0.6%,
    int8 att ~0.96%, bf16 weights/e ~0.3%; gate is 2e-2.

Per-iteration engine streams (6-stage skewed software pipeline; exp sits
two iterations behind scores so ACT, which runs ~2us/iter faster than
PE, never blocks the tanh queued behind it):
    ACT:  4x tanh(b), exp(b-3)
    PE:   Z(b-4), 16x wsum(b-4), 32x scores(b-1)
    DVE:  recip(b-4), 2x out-scale(b-5)
    gpsimd: p-group + att-slab cast-DMA issues (consumption order)
    sync: consts, out store (b-5)
"""

import numpy as np

B, R, K_H, D, F = 128, 1024, 1024, 512, 1024
N_CORES = 8
BPC = B // N_CORES  # batches per core
RC = R // 128  # region chunks (r = 8*rp + j)
DC = D // 128  # d chunks (d = 4*pp + dc)
KC = K_H // 128
S_P = 4.0 / 127.0  # p_att_feats int8 scale
S_A = 4.0 / 127.0  # att_feats int8 scale
AB = 2  # batches per att slab
PB = 4  # batches per p slab

_cached = {}


def _build_program():
    from contextlib import ExitStack

    import concourse.bass as bass
    import concourse.tile as tile
    from concourse import bacc, mybir

    f32 = mybir.dt.float32
    bf16 = mybir.dt.bfloat16
    i8 = mybir.dt.int8
    AF = mybir.ActivationFunctionType
    ALU = mybir.AluOpType

    nc = bacc.Bacc("TRN2", target_bir_lowering=False, debug=False)
    p_ap = nc.dram_tensor(
        "p", [BPC // PB, 128, DC * PB * R], i8, kind="ExternalInput"
    ).ap()
    att_ap = nc.dram_tensor(
        "att", [128, BPC * RC * F], i8, kind="ExternalInput"
    ).ap()
    w_ap = nc.dram_tensor("w_h2att", [128, KC * D], bf16, kind="ExternalInput").ap()
    ht_ap = nc.dram_tensor("ht", [128, KC * BPC], bf16, kind="ExternalInput").ap()
    cst_ap = nc.dram_tensor("csts", [128, 24], mybir.dt.uint8, kind="ExternalInput").ap()
    out_ap = nc.dram_tensor("out", [BPC, F], f32, kind="ExternalOutput").ap()

    with tile.TileContext(nc) as tc, ExitStack() as ctx:
        consts = ctx.enter_context(tc.tile_pool(name="consts", bufs=1))
        ppool = ctx.enter_context(tc.tile_pool(name="ppool", bufs=2))
        epool = ctx.enter_context(tc.tile_pool(name="epool", bufs=2))
        apool = ctx.enter_context(tc.tile_pool(name="apool", bufs=4))
        spool = ctx.enter_context(tc.tile_pool(name="spool", bufs=6))
        outp = ctx.enter_context(tc.tile_pool(name="outp", bufs=2))
        ps_mis = ctx.enter_context(tc.tile_pool(name="ps_mis", bufs=3, space="PSUM"))
        ps_out = ctx.enter_context(tc.tile_pool(name="ps_out", bufs=2, space="PSUM"))
        ps_z = ctx.enter_context(tc.tile_pool(name="ps_z", bufs=1, space="PSUM"))

        p_tiles, a_tiles = {}, {}

        def issue_p(g):
            # [128(pp), 4(dc), PB(k), 1024(col)]; d = 4*pp+dc, col = j*128+rp
            t = ppool.tile([128, DC, PB, R], i8, tag="p")
            nc.gpsimd.dma_start(out=t, in_=p_ap[g])
            p_tiles[g] = t

        # variable-width slabs: 2-batch mid-stream (16KB int8 runs, best
        # rate), single batches at the end so tail compute overlaps the
        # last arriving bytes
        SL = [(0, 2), (2, 2), (4, 2), (6, 2), (8, 2), (10, 2), (12, 2), (14, 1), (15, 1)]
        b2slab = {}
        for si, (s0, nb) in enumerate(SL):
            for k in range(nb):
                b2slab[s0 + k] = (si, k)

        def issue_a(si):
            s0, nb = SL[si]
            # int8 HBM -> bf16 SBUF cast during the DMA (SWDGE only)
            t = apool.tile([128, nb, RC, F], bf16, tag="a", padded_shape=[128, AB, RC, F])
            nc.gpsimd.dma_start(
                out=t, in_=att_ap[:, s0 * RC * F : (s0 + nb) * RC * F].rearrange(
                    "p (k j f) -> p k j f", k=nb, j=RC
                )
            )
            a_tiles[si] = t

        # W first on the gpsimd queue: FIFO means it fully drains before
        # the att slabs instead of round-robining behind the prefetch burst.
        w_flat = consts.tile([128, KC * D], bf16, tag="w")
        nc.gpsimd.dma_start(out=w_flat, in_=w_ap)
        issue_p(0)
        cst_t = consts.tile([128, 24], mybir.dt.uint8, tag="csts")
        nc.sync.dma_start(out=cst_t, in_=cst_ap)
        wa_t = cst_t[:, 0:8].bitcast(bf16)  # [128, 4]
        b2_t = cst_t[:, 8:24].bitcast(f32)  # [128, 4]
        ht_flat = consts.tile([128, KC * BPC], bf16, tag="ht")
        nc.sync.dma_start(out=ht_flat, in_=ht_ap)
        issue_p(1)
        for g in range(4):
            issue_a(g)
        ones_t = consts.tile([128, 1], f32, tag="ones")
        nc.vector.memset(ones_t, 1.0)

        # ---- phase 0: h_attT[d, b] = (h @ W + b2).T, kept [128, 4, 16] f32
        hattT = consts.tile([128, DC, BPC], f32, tag="hatt")
        for mc in range(DC):
            ps_h = ps_mis.tile([128, BPC], f32, tag="mis")
            for kc in range(KC):
                nc.tensor.matmul(
                    ps_h,
                    lhsT=w_flat[:, kc * D + mc * 128 : kc * D + (mc + 1) * 128],
                    rhs=ht_flat[:, kc * BPC : (kc + 1) * BPC],
                    start=(kc == 0),
                    stop=(kc == KC - 1),
                )
            nc.scalar.activation(
                hattT[:, mc, :], ps_h, AF.Identity, bias=b2_t[:, mc : mc + 1]
            )

        # ---- main loop: 5-stage skewed software pipeline
        e_t, sc_t, ex_t, s1_t, z_t, rz_t, po_t = {}, {}, {}, {}, {}, {}, {}
        for i in range(BPC + 5):
            # T1 (ACT): tanh for b, first in the ACT stream (its inputs are
            # ready at iteration start; exp(b-2) depends on PE's tail)
            if i < BPC:
                b = i
                p_t = p_tiles[b // PB]
                e_tt = epool.tile([128, DC, R], bf16, tag="e")
                for dc in range(DC):
                    nc.scalar.activation(
                        e_tt[:, dc, :],
                        p_t[:, dc, b % PB, :],
                        AF.Tanh,
                        bias=hattT[:, dc, b : b + 1],
                        scale=S_P,
                    )
                if b % PB == PB - 1:
                    # next p group: its slot's last consumer just ran here
                    p_tiles.pop(b // PB)
                    if b // PB + 2 < BPC // PB:
                        issue_p(b // PB + 2)
                e_t[b] = e_tt

            # T3 (ACT): exp of scores(b-3) — two iterations of slack after
            # PE's scores so ACT never blocks even when it drifts ahead
            if 0 <= i - 3 < BPC:
                b = i - 3
                expb = spool.tile([128, RC], bf16, tag="expb")
                s1 = spool.tile([128, 1], f32, tag="s1")
                nc.scalar.activation(expb, sc_t.pop(b), AF.Exp, accum_out=s1)
                ex_t[b], s1_t[b] = expb, s1

            # T4 (PE): Z partition-sum + weighted sum for b-4
            if 0 <= i - 4 < BPC:
                b = i - 4
                ps_zz = ps_z.tile([1, 1], f32, tag="z")
                nc.tensor.matmul(
                    ps_zz, lhsT=s1_t.pop(b), rhs=ones_t, start=True, stop=True
                )
                z_t[b] = ps_zz
                si, kk = b2slab[b]
                a_tt = a_tiles[si]
                expb = ex_t.pop(b)
                ps0 = ps_out.tile([1, 512], f32, tag="ps0")
                ps1 = ps_out.tile([1, 512], f32, tag="ps1")
                for j in range(RC):
                    nc.tensor.matmul(
                        ps0,
                        lhsT=expb[:, j : j + 1],
                        rhs=a_tt[:, kk, j, 0:512],
                        start=(j == 0),
                        stop=(j == RC - 1),
                    )
                    nc.tensor.matmul(
                        ps1,
                        lhsT=expb[:, j : j + 1],
                        rhs=a_tt[:, kk, j, 512:1024],
                        start=(j == 0),
                        stop=(j == RC - 1),
                    )
                po_t[b] = (ps0, ps1)
                if kk == SL[si][1] - 1:
                    # slab finished: refill its slot in consumption order
                    a_tiles.pop(si)
                    if si + 4 < len(SL):
                        issue_a(si + 4)

            # T5 (DVE): 1/Z for b-4; out-scale + store for b-5
            if 0 <= i - 4 < BPC:
                b = i - 4
                rz = spool.tile([1, 1], f32, tag="rz")
                nc.vector.reciprocal(rz, z_t.pop(b))
                rz_t[b] = rz
            if 0 <= i - 5 < BPC:
                b = i - 5
                ps0, ps1 = po_t.pop(b)
                rz = rz_t.pop(b)
                ob = outp.tile([1, F], f32)
                nc.vector.tensor_scalar(
                    ob[:, 0:512], ps0, rz, S_A, op0=ALU.mult, op1=ALU.mult
                )
                nc.vector.tensor_scalar(
                    ob[:, 512:1024], ps1, rz, S_A, op0=ALU.mult, op1=ALU.mult
                )
                nc.sync.dma_start(out=out_ap[b : b + 1, :], in_=ob)

            # T2 (PE): scores(b-1) as [128(rp), 8(j)] via stationary-e matmuls
            if 0 <= i - 1 < BPC:
                b = i - 1
                e_tt = e_t.pop(b)
                ps_s = ps_mis.tile([128, RC], f32, tag="mis")
                for j in range(RC):
                    for dc in range(DC):
                        nc.tensor.matmul(
                            ps_s[:, j : j + 1],
                            lhsT=e_tt[:, dc, j * 128 : (j + 1) * 128],
                            rhs=wa_t[:, dc : dc + 1],
                            start=(dc == 0),
                            stop=(dc == DC - 1),
                        )
                sc_t[b] = ps_s

    nc.compile()
    return nc


def _get_program():
    if "nc" not in _cached:
        _cached["nc"] = _build_program()
    return _cached["nc"]


def _make_in_maps(inputs):
    import ml_dtypes

    bf = ml_dtypes.bfloat16
    h = np.asarray(inputs["h"], dtype=np.float32)
    att = np.asarray(inputs["att_feats"], dtype=np.float32)
    p = np.asarray(inputs["p_att_feats"], dtype=np.float32)
    W = np.asarray(inputs["W_h2att"], dtype=np.float32)
    b2 = np.asarray(inputs["b_h2att"], dtype=np.float32)
    wa = np.asarray(inputs["w_alpha"], dtype=np.float32)
    # b_alpha is a scalar added to every score; softmax is shift-invariant.

    # p: [B, R, D] -> int8 [B, D, R'] with r' = j*128 + rp for r = 8*rp + j
    pq = np.clip(np.rint(p * (1.0 / S_P)), -127, 127).astype(np.int8)
    pT = np.ascontiguousarray(
        pq.reshape(B, 128, RC, D).transpose(0, 3, 2, 1)
    ).reshape(B, D, R)
    # att: int8, 2-batch slabs, per-partition contiguous [g, 128, 2*8*1024]
    aq = np.clip(np.rint(att * (1.0 / S_A)), -127, 127).astype(np.int8)
    # W columns permuted so matmul chunk mc / partition pp -> d = 4*pp + mc,
    # then packed per-partition: W_pack[pk, kc*D + m] = W_perm[kc*128+pk, m]
    Wp = np.ascontiguousarray(
        W.reshape(K_H, 128, DC).transpose(0, 2, 1)
    ).reshape(K_H, D).astype(bf)
    W_pack = np.ascontiguousarray(
        Wp.reshape(KC, 128, D).transpose(1, 0, 2)
    ).reshape(128, KC * D)
    hT = np.ascontiguousarray(h.T).astype(bf)  # [K_H, B]
    wa_t = np.ascontiguousarray(wa.reshape(128, DC)).astype(bf)
    b2_t = np.ascontiguousarray(b2.reshape(128, DC)).astype(np.float32)
    csts = np.ascontiguousarray(
        np.concatenate([wa_t.view(np.uint8), b2_t.view(np.uint8)], axis=1)
    )  # [128, 24]

    in_maps = []
    for c in range(N_CORES):
        lo, hi = c * BPC, (c + 1) * BPC
        # p groups of PB batches: [g, 128, DC*PB*R] per-partition contiguous
        p4 = np.ascontiguousarray(
            pT[lo:hi]
            .reshape(BPC // PB, PB, 128, DC, R)  # d = pp*4 + dc
            .transpose(0, 2, 3, 1, 4)  # [g, pp, dc, k, R]
        ).reshape(BPC // PB, 128, DC * PB * R)
        # att flat: [128, BPC*RC*F]; per partition batch-major, r = 8*rp+j
        a2 = np.ascontiguousarray(
            aq[lo:hi].reshape(BPC, 128, RC * F).transpose(1, 0, 2)
        ).reshape(128, BPC * RC * F)
        # ht_pack[pk, kc*BPC + b] = h[lo+b, kc*128+pk]
        ht_pack = np.ascontiguousarray(
            hT[:, lo:hi].reshape(KC, 128, BPC).transpose(1, 0, 2)
        ).reshape(128, KC * BPC)
        in_maps.append(
            {
                "p": p4,
                "att": a2,
                "w_h2att": W_pack,
                "ht": ht_pack,
                "csts": csts,
            }
        )
    return in_maps


def kernel(**inputs) -> np.ndarray:
    from concourse.bass_utils import run_bass_kernel_spmd

    nc = _get_program()
    in_maps = _make_in_maps(inputs)
    res = run_bass_kernel_spmd(nc, in_maps, list(range(N_CORES)))
    out = np.concatenate([res.results[c]["out"] for c in range(N_CORES)], axis=0)
    return out.astype(np.float32)
